# revision 14
# baseline (speedup 1.0000x reference)
"""Distributed causal multi-head attention on one TRN2 chip (8 NeuronCores).

Problem: B=2, S=2048, D=1024, H=16, DH=64 (f32), causal softmax attention with
QKV + output projections.

Sharding (SPMD, one Bass graph for all 8 cores):
  core i -> batch b = i // 4, head group g = i % 4 (4 of 16 heads).
Each core projects Q/K/V for its 4 heads over the full sequence of its batch
and runs causal attention.  Per-head z (bf16) is AllGathered within each
batch's 4-core group one 512-row band at a time; each core then computes a
256-column slice of the output projection.  Core (b, g) returns
out[b, :, 256g:256g+256]; the host concatenates.

Schedule: a single software-pipelined band loop.  Attention of band t is
interleaved (at matmul-quantum granularity, via generators) with the QKV
projections of band t+1 and the output projection of band t-1, so the PE
queue never drains while ACT runs the exps.  The exp is batched over a
2-bank PSUM pair (both heads of a pair-packed group), diagonal chunks run
restricted q-ranges, and causal masking is a multiplicative bf16 0/1 mask
applied post-exp on the DVE.  Scores matmuls are row-tiled (contract=64,
two heads concurrently on PE row-groups 0-63/64-127).  The softmax
denominator rides a ones-column in V; its reciprocal uses the fast custom
DVE op and is broadcast across partitions on the idle GpSimd engine.
The final band's gather + output projection is split in two q-halves to
shorten the tail.
"""

import sys

for _p in ("/opt/trn_rl_repo", "/opt/pypackages"):
    if _p not in sys.path:
        sys.path.insert(0, _p)

from contextlib import ExitStack

import numpy as np

import concourse.bass as bass
import concourse.mybir as mybir
import concourse.tile as tile
from concourse import bacc
from concourse.bass_utils import run_bass_kernel_spmd

B, S, D, H, DH = 2, 2048, 1024, 16, 64
G = 4                       # heads per core
NCORES = 8
SCALE = float(np.sqrt(DH))
TQ = 512                    # query band (free dim)
NQT = S // TQ               # 4
KC = 128                    # key chunk (partition dim)
DC = 128                    # contraction d-chunk
NDC = D // DC               # 8
EG = G * DH                 # 256: packed head dim per group
VW = DH + 1                 # 65: head slot width in v_aug (ones column)
GW = G * VW                 # 260: v_aug width per k-chunk
DS = D // 4                 # 256: output D-column slice per core

F32 = mybir.dt.float32
BF16 = mybir.dt.bfloat16
EXP = mybir.ActivationFunctionType.Exp
MULT = mybir.AluOpType.mult

GROUPS = [[0, 1, 2, 3], [4, 5, 6, 7]]

USE_GPSIMD_BCAST = False
USE_FAST_RECIP = False

_CACHE = {}


def _build() -> bass.Bass:
    nc = bacc.Bacc("TRN2", num_devices=NCORES, target_bir_lowering=False)

    xq = nc.declare_dram_parameter("xq", [D, S], BF16, isOutput=False)
    xk = nc.declare_dram_parameter("xk", [D, S], BF16, isOutput=False)
    xv = nc.declare_dram_parameter("xv", [D, S], BF16, isOutput=False)
    wq = nc.declare_dram_parameter("wq", [NDC, DC, EG], BF16, isOutput=False)
    wk = nc.declare_dram_parameter("wk", [NDC, DC, EG], BF16, isOutput=False)
    wv = nc.declare_dram_parameter("wv", [NDC, DC, EG], BF16, isOutput=False)
    wo = nc.declare_dram_parameter("wo", [NDC, DC, DS], BF16, isOutput=False)
    mask = nc.declare_dram_parameter("mask", [KC, G * TQ], BF16, isOutput=False)
    out_ext = nc.declare_dram_parameter("out", [S, DS], F32, isOutput=True)

    with ExitStack() as ctx:
        tc = ctx.enter_context(tile.TileContext(nc))
        const = ctx.enter_context(tc.tile_pool(name="const", bufs=1))
        dram = ctx.enter_context(tc.tile_pool(name="dram", bufs=1, space="DRAM"))
        xpool = ctx.enter_context(tc.tile_pool(name="x", bufs=2))
        epool = ctx.enter_context(tc.tile_pool(name="e", bufs=3))
        rpool = ctx.enter_context(tc.tile_pool(name="r", bufs=2))
        zgpool = ctx.enter_context(tc.tile_pool(name="zg", bufs=2))
        opool = ctx.enter_context(tc.tile_pool(name="o", bufs=2))
        # PSUM: pz 4 banks + sc pair 2 banks + proj 2 banks = 8
        pzpool = ctx.enter_context(tc.tile_pool(name="pz", bufs=1, space="PSUM"))
        scpool = ctx.enter_context(tc.tile_pool(name="sc", bufs=1, space="PSUM"))
        prpool = ctx.enter_context(tc.tile_pool(name="pr", bufs=2, space="PSUM"))

        # ---- persistent SBUF tiles ----
        wq_sb = const.tile([DC, NDC * EG], BF16, name="wq_sb")
        wk_sb = const.tile([DC, NDC * EG], BF16, name="wk_sb")
        wv_sb = const.tile([DC, NDC * EG], BF16, name="wv_sb")
        wo_sb = const.tile([DC, NDC * DS], BF16, name="wo_sb")
        mask_sb = const.tile([KC, G * TQ], BF16, name="mask_sb")
        vaug = const.tile([KC, (S // KC) * GW], BF16, name="vaug")
        q_sb = [const.tile([2 * DH, S], BF16, name=f"q_sb{p}") for p in range(2)]
        k_sb = [const.tile([2 * DH, S], BF16, name=f"k_sb{p}") for p in range(2)]
        z_sb = [const.tile([2 * DH, S], BF16, name=f"z_sb{p}") for p in range(2)]

        # ---- DRAM staging for the z AllGather ----
        # bands 0..2: full-band gathers; band 3: two q-halves to cut the tail
        zb = [dram.tile([EG, TQ], BF16, name=f"zb{t}") for t in range(3)]
        zg = [dram.tile([G * EG, TQ], BF16, name=f"zg{t}") for t in range(3)]
        zbh = [dram.tile([EG, TQ // 2], BF16, name=f"zbh{i}") for i in range(2)]
        zgh = [
            dram.tile([G * EG, TQ // 2], BF16, name=f"zgh{i}") for i in range(2)
        ]
        dgi = dram.tile([1, 64], BF16, name="dgi")
        dgo = dram.tile([1, 256], BF16, name="dgo")

        # ---- prologue ----
        # dummy AllGather first: absorbs collective-stream init + rendezvous
        # cost so the first real gather is cheap
        dgz = const.tile([1, 64], BF16, name="dgz")
        nc.vector.memset(dgz[:], 0.0)
        ones_f32 = const.tile([1, DH], F32, name="ones_f32")
        nc.vector.memset(ones_f32[:], 1.0)
        nc.gpsimd.dma_start(dgi[:, :], dgz[:])
        nc.gpsimd.collective_compute(
            "AllGather",
            mybir.AluOpType.bypass,
            replica_groups=GROUPS,
            ins=[dgi.opt()],
            outs=[dgo.opt()],
        )
        # ones base for v_aug (value slots get overwritten by the v copies)
        nc.vector.memset(vaug[:], 1.0)

        x_t = {}

        def load_x_band(t):
            for nm, src in (("q", xq), ("k", xk), ("v", xv)):
                xt = xpool.tile([DC, NDC * TQ], BF16, name=f"x{nm}", tag=f"x{nm}")
                src_v = src[:, :].rearrange("(c p) s -> p c s", p=DC)[
                    :, :, t * TQ : (t + 1) * TQ
                ]
                nc.sync.dma_start(
                    xt[:].rearrange("p (c s) -> p c s", c=NDC), src_v
                )
                x_t[(nm, t)] = xt

        # band-0 activations first (they gate the first matmuls), then weights
        load_x_band(0)
        for wsrc, wdst in ((wq, wq_sb), (wk, wk_sb), (wv, wv_sb), (wo, wo_sb)):
            nc.sync.dma_start(
                wdst[:].rearrange("p (c e) -> p c e", c=NDC),
                wsrc[:, :, :].rearrange("c p e -> p c e"),
            )
        nc.sync.dma_start(mask_sb[:], mask[:, :])

        # ---- QKV projection of one band (generator: yields between quanta) ----
        def proj_band_gen(t):
            if t > 0:
                load_x_band(t)
                yield
            xq_t, xk_t, xv_t = x_t[("q", t)], x_t[("k", t)], x_t[("v", t)]
            for xt, wsb, dst in ((xq_t, wq_sb, q_sb), (xk_t, wk_sb, k_sb)):
                for p in range(2):
                    pq = prpool.tile([DC, TQ], F32, tag="pr", name="pq")
                    for c in range(NDC):
                        nc.tensor.matmul(
                            pq[:],
                            wsb[:, c * EG + p * DC : c * EG + (p + 1) * DC],
                            xt[:, c * TQ : (c + 1) * TQ],
                            start=(c == 0),
                            stop=(c == NDC - 1),
                        )
                        if c % 4 == 3:
                            yield
                    nc.vector.tensor_copy(
                        dst[p][:, t * TQ : (t + 1) * TQ], pq[:]
                    )
            for sub in range(4):
                pv = prpool.tile([DC, TQ], F32, tag="pr", name="pv")
                for c in range(NDC):
                    nc.tensor.matmul(
                        pv[:, 0:EG],
                        xv_t[:, c * TQ + sub * KC : c * TQ + (sub + 1) * KC],
                        wv_sb[:, c * EG : (c + 1) * EG],
                        start=(c == 0),
                        stop=(c == NDC - 1),
                    )
                    if c % 4 == 3:
                        yield
                kci = t * 4 + sub
                base = kci * GW
                nc.vector.tensor_copy(
                    vaug[:, base : base + GW].rearrange(
                        "p (h w) -> p h w", h=G
                    )[:, :, 0:DH],
                    pv[:, 0:EG].rearrange("p (h e) -> p h e", h=G),
                )
            yield

        # ---- output projection of one band (generator) ----
        def oproj_band_gen(t, zgt, q0, nq):
            # out[t*TQ+q0 : +nq, :] = z_all^T @ W_O[:, cols of this core]
            nqs = nq // KC
            zg_sb = zgpool.tile([DC, NDC * nq], BF16, name="zg_sb", tag="zg")
            nc.gpsimd.dma_start(
                zg_sb[:].rearrange("p (c s) -> p c s", c=NDC),
                zgt[:, :].rearrange("(c p) s -> p c s", p=DC),
            )
            yield
            o_sb = opool.tile([DC, nqs * DS], F32, name="o_sb", tag="o")
            for half in range(nqs // 2):
                po = prpool.tile([DC, 2 * DS], F32, tag="pr", name="po")
                for sub in range(2):
                    qs = half * 2 + sub
                    for c in range(NDC):
                        nc.tensor.matmul(
                            po[:, sub * DS : (sub + 1) * DS],
                            zg_sb[:, c * nq + qs * KC : c * nq + (qs + 1) * KC],
                            wo_sb[:, c * DS : (c + 1) * DS],
                            start=(c == 0),
                            stop=(c == NDC - 1),
                        )
                        if c % 4 == 3:
                            yield
                nc.vector.tensor_copy(
                    o_sb[:, half * 2 * DS : (half + 1) * 2 * DS], po[:]
                )
            nc.gpsimd.dma_start(
                out_ext[:, :].rearrange("(b p) d -> p b d", p=KC)[
                    :, 4 * t + q0 // KC : 4 * t + q0 // KC + nqs, :
                ],
                o_sb[:].rearrange("p (b d) -> p b d", b=nqs),
            )
            yield

        def pull(it, n):
            for _ in range(n):
                try:
                    next(it)
                except StopIteration:
                    return

        def drain(it):
            for _ in it:
                pass

        # ---- normalize + stage + gather a q-range of band t ----
        def make_recip_bcast(pz):
            den_r = rpool.tile([1, G * TQ], F32, tag="denr", name="den_r")
            bc = rpool.tile([DH, G * TQ], F32, tag="bc", name="bc")
            if USE_FAST_RECIP:
                nc.vector.reciprocal_approx_fast(den_r[:], pz[DH : DH + 1, :])
            else:
                nc.vector.reciprocal(den_r[:], pz[DH : DH + 1, :])
            if USE_GPSIMD_BCAST:
                nc.gpsimd.partition_broadcast(bc[:], den_r[:], channels=DH)
            else:
                # rank-1 broadcast of 1/den on the PE
                for h in range(G):
                    pb = prpool.tile([DC, TQ], F32, tag="pr", name="pb")
                    nc.tensor.matmul(
                        pb[0:DH, :],
                        ones_f32[:],
                        den_r[:, h * TQ : (h + 1) * TQ],
                        start=True,
                        stop=True,
                    )
                    nc.vector.tensor_copy(bc[:, h * TQ : (h + 1) * TQ], pb[0:DH, :])
            return bc

        def norm_stage_gather(t, pz, bc, q0, nq, zbt, zgt):
            for h in range(G):
                p_i, off = h // 2, (h % 2) * DH
                nc.vector.tensor_mul(
                    z_sb[p_i][off : off + DH, t * TQ + q0 : t * TQ + q0 + nq],
                    pz[0:DH, h * TQ + q0 : h * TQ + q0 + nq],
                    bc[:, h * TQ + q0 : h * TQ + q0 + nq],
                )
            for p in range(2):
                nc.gpsimd.dma_start(
                    zbt[p * KC : (p + 1) * KC, :],
                    z_sb[p][:, t * TQ + q0 : t * TQ + q0 + nq],
                )
            nc.gpsimd.collective_compute(
                "AllGather",
                mybir.AluOpType.bypass,
                replica_groups=GROUPS,
                ins=[zbt.opt()],
                outs=[zgt.opt()],
            )

        # ---- attention band with interleaved background work ----
        def attention_band(t, work):
            nkc = 4 * (t + 1)
            pz = pzpool.tile([VW, G * TQ], F32, tag="pz", name="pz")
            for kci in range(nkc):
                dc = kci - 4 * t
                qv = KC * dc if dc >= 0 else 0
                nq = TQ - qv
                es = []
                for pair in range(2):
                    sc = scpool.tile([KC, 2 * TQ], F32, tag="sc", name="sc")
                    for h2 in range(2):
                        nc.tensor.matmul(
                            sc[:, h2 * TQ + qv : (h2 + 1) * TQ],
                            k_sb[pair][
                                h2 * DH : (h2 + 1) * DH,
                                kci * KC : (kci + 1) * KC,
                            ],
                            q_sb[pair][
                                h2 * DH : (h2 + 1) * DH,
                                t * TQ + qv : (t + 1) * TQ,
                            ],
                            start=True,
                            stop=True,
                            tile_position=(h2 * DH, 0),
                        )
                    e = epool.tile([KC, 2 * TQ], BF16, tag=f"e{pair}", name="e")
                    sc_v = sc[:].rearrange("p (h q) -> p h q", h=2)[:, :, qv:TQ]
                    e_v = e[:].rearrange("p (h q) -> p h q", h=2)[:, :, qv:TQ]
                    nc.scalar.activation(e_v, sc_v, EXP)
                    if dc >= 0:
                        m_v = (
                            mask_sb[:, dc * TQ + qv : (dc + 1) * TQ]
                            .unsqueeze(1)
                            .broadcast_to([KC, 2, nq])
                        )
                        nc.vector.tensor_mul(e_v, e_v, m_v)
                    es.append(e)
                for h in range(G):
                    e = es[h // 2]
                    h2 = h % 2
                    nc.tensor.matmul(
                        pz[:, h * TQ + qv : (h + 1) * TQ],
                        vaug[:, kci * GW + h * VW : kci * GW + (h + 1) * VW],
                        e[:, h2 * TQ + qv : (h2 + 1) * TQ],
                        start=(kci == 0),
                        stop=(kci == nkc - 1),
                    )
                pull(work, 2)
            return pz

        # ---- the pipeline ----
        drain(proj_band_gen(0))
        ogens = {}
        for t in range(NQT):
            gens = []
            if t + 1 < NQT:
                gens.append(proj_band_gen(t + 1))
            if t - 1 >= 0:
                gens.append(ogens[t - 1])
            work = _chain(gens)
            pz = attention_band(t, work)
            drain(work)
            bc = make_recip_bcast(pz)
            if t < 3:
                norm_stage_gather(t, pz, bc, 0, TQ, zb[t], zg[t])
                ogens[t] = oproj_band_gen(t, zg[t], 0, TQ)
            else:
                for i in range(2):
                    norm_stage_gather(
                        t, pz, bc, i * (TQ // 2), TQ // 2, zbh[i], zgh[i]
                    )
                for i in range(2):
                    drain(oproj_band_gen(t, zgh[i], i * (TQ // 2), TQ // 2))

    nc.compile()
    return nc


def _chain(gens):
    for g in gens:
        yield from g


def _get_graph() -> bass.Bass:
    if "nc" not in _CACHE:
        _CACHE["nc"] = _build()
    return _CACHE["nc"]


def _make_mask() -> np.ndarray:
    import ml_dtypes

    m = np.empty((KC, G * TQ), np.float32)
    x = np.arange(KC)[:, None]
    y = np.arange(TQ)[None, :]
    for dc in range(G):
        m[:, dc * TQ : (dc + 1) * TQ] = (dc * KC + x <= y).astype(np.float32)
    return m.astype(ml_dtypes.bfloat16)


def _make_in_maps(inputs: dict) -> list[dict]:
    import ml_dtypes

    bf16 = ml_dtypes.bfloat16
    qx = np.asarray(inputs["query_input"], np.float32).astype(bf16)
    kx = np.asarray(inputs["key_input"], np.float32).astype(bf16)
    vx = np.asarray(inputs["value_input"], np.float32).astype(bf16)
    # fold the attention scale into W_Q on the host
    WQ = (np.asarray(inputs["W_Q"], np.float32) / SCALE).astype(bf16)
    WK = np.asarray(inputs["W_K"], np.float32).astype(bf16)
    WV = np.asarray(inputs["W_V"], np.float32).astype(bf16)
    WO = np.asarray(inputs["W_O"], np.float32).astype(bf16)

    mask = _make_mask()
    xT = {
        (nm, b): np.ascontiguousarray(arr[b].T)
        for nm, arr in (("xq", qx), ("xk", kx), ("xv", vx))
        for b in range(B)
    }
    WO_flat = WO.reshape(H * DH, D)  # e' = h*64 + e, h-major (AllGather order)
    wmaps = []
    for g in range(G):
        hs = slice(g * G, (g + 1) * G)

        def prep(w):
            return np.ascontiguousarray(
                w[hs].transpose(1, 0, 2).reshape(D, EG).reshape(NDC, DC, EG)
            )

        wmaps.append(
            {
                "wq": prep(WQ),
                "wk": prep(WK),
                "wv": prep(WV),
                "wo": np.ascontiguousarray(
                    WO_flat[:, g * DS : (g + 1) * DS].reshape(NDC, DC, DS)
                ),
            }
        )

    in_maps = []
    for core in range(NCORES):
        b, g = core // G, core % G
        m = {
            "xq": xT[("xq", b)],
            "xk": xT[("xk", b)],
            "xv": xT[("xv", b)],
            "mask": mask,
        }
        m.update(wmaps[g])
        in_maps.append(m)
    return in_maps


def _assemble(results: list[dict]) -> np.ndarray:
    out = np.empty((B, S, D), np.float32)
    for core in range(NCORES):
        b, g = core // G, core % G
        out[b, :, g * DS : (g + 1) * DS] = results[core]["out"]
    return out


def run(inputs: dict, trace: bool = False):
    """Run on hardware; returns (output, BassKernelResults)."""
    nc = _get_graph()
    res = run_bass_kernel_spmd(
        nc, _make_in_maps(inputs), core_ids=list(range(NCORES)), trace=trace
    )
    return _assemble(res.results), res


def kernel(**inputs) -> np.ndarray:
    out, _ = run(inputs)
    return out


# revision 16
# speedup vs baseline: 1.2588x; 1.2588x over previous
"""Distributed causal multi-head attention on one TRN2 chip (8 NeuronCores).

Problem: B=2, S=2048, D=1024, H=16, DH=64 (f32), causal softmax attention with
QKV + output projections.

Sharding (SPMD, one Bass graph for all 8 cores):
  core i -> batch b = i // 4, head group g = i % 4 (4 of 16 heads).
Each core projects Q/K/V for its 4 heads over the full sequence of its batch
and runs causal attention.  Per-head z (bf16) is AllGathered within each
batch's 4-core group one 512-row band at a time; each core then computes a
256-column slice of the output projection.  Core (b, g) returns
out[b, :, 256g:256g+256]; the host concatenates.

Schedule: a single software-pipelined band loop.  Attention of band t is
interleaved (at matmul-quantum granularity, via generators) with the QKV
projections of band t+1 and the output projection of band t-1, so the PE
queue never drains while ACT runs the exps.  The exp is batched over a
2-bank PSUM pair (both heads of a pair-packed group), diagonal chunks run
restricted q-ranges, and causal masking is a multiplicative bf16 0/1 mask
applied post-exp on the DVE.  Scores matmuls are row-tiled (contract=64,
two heads concurrently on PE row-groups 0-63/64-127).  The softmax
denominator rides a ones-column in V; its reciprocal uses the fast custom
DVE op and is broadcast across partitions on the idle GpSimd engine.
The final band's gather + output projection is split in two q-halves to
shorten the tail.
"""

import sys

for _p in ("/opt/trn_rl_repo", "/opt/pypackages"):
    if _p not in sys.path:
        sys.path.insert(0, _p)

from contextlib import ExitStack

import numpy as np

import concourse.bass as bass
import concourse.mybir as mybir
import concourse.tile as tile
from concourse import bacc
from concourse.bass_utils import run_bass_kernel_spmd

B, S, D, H, DH = 2, 2048, 1024, 16, 64
G = 4                       # heads per core
NCORES = 8
SCALE = float(np.sqrt(DH))
TQ = 512                    # query band (free dim)
NQT = S // TQ               # 4
KC = 128                    # key chunk (partition dim)
DC = 128                    # contraction d-chunk
NDC = D // DC               # 8
EG = G * DH                 # 256: packed head dim per group
VW = DH + 1                 # 65: head slot width in v_aug (ones column)
GW = G * VW                 # 260: v_aug width per k-chunk
DS = D // 4                 # 256: output D-column slice per core

F32 = mybir.dt.float32
BF16 = mybir.dt.bfloat16
EXP = mybir.ActivationFunctionType.Exp
LN = mybir.ActivationFunctionType.Ln
MULT = mybir.AluOpType.mult

GROUPS = [[0, 1, 2, 3], [4, 5, 6, 7]]

USE_GPSIMD_BCAST = False
USE_FAST_RECIP = False

_CACHE = {}


def _build() -> bass.Bass:
    nc = bacc.Bacc("TRN2", num_devices=NCORES, target_bir_lowering=False)

    xq = nc.declare_dram_parameter("xq", [D, S], BF16, isOutput=False)
    xk = nc.declare_dram_parameter("xk", [D, S], BF16, isOutput=False)
    xv = nc.declare_dram_parameter("xv", [D, S], BF16, isOutput=False)
    wq = nc.declare_dram_parameter("wq", [NDC, DC, EG], BF16, isOutput=False)
    wk = nc.declare_dram_parameter("wk", [NDC, DC, EG], BF16, isOutput=False)
    wv = nc.declare_dram_parameter("wv", [NDC, DC, EG], BF16, isOutput=False)
    wo = nc.declare_dram_parameter("wo", [NDC, DC, DS], BF16, isOutput=False)
    mask = nc.declare_dram_parameter("mask", [KC, G * TQ], BF16, isOutput=False)
    out_ext = nc.declare_dram_parameter("out", [S, DS], F32, isOutput=True)

    with ExitStack() as ctx:
        tc = ctx.enter_context(tile.TileContext(nc))
        const = ctx.enter_context(tc.tile_pool(name="const", bufs=1))
        dram = ctx.enter_context(tc.tile_pool(name="dram", bufs=1, space="DRAM"))
        xpool = ctx.enter_context(tc.tile_pool(name="x", bufs=2))
        epool = ctx.enter_context(tc.tile_pool(name="e", bufs=3))
        rpool = ctx.enter_context(tc.tile_pool(name="r", bufs=2))
        zgpool = ctx.enter_context(tc.tile_pool(name="zg", bufs=2))
        opool = ctx.enter_context(tc.tile_pool(name="o", bufs=2))
        # PSUM: pz 4 banks + sc pair 2 banks + proj 2 banks = 8
        pzpool = ctx.enter_context(tc.tile_pool(name="pz", bufs=1, space="PSUM"))
        scpool = ctx.enter_context(tc.tile_pool(name="sc", bufs=1, space="PSUM"))
        prpool = ctx.enter_context(tc.tile_pool(name="pr", bufs=2, space="PSUM"))

        # ---- persistent SBUF tiles ----
        wq_sb = const.tile([DC, NDC * EG], BF16, name="wq_sb")
        wk_sb = const.tile([DC, NDC * EG], BF16, name="wk_sb")
        wv_sb = const.tile([DC, NDC * EG], BF16, name="wv_sb")
        wo_sb = const.tile([DC, NDC * DS], BF16, name="wo_sb")
        mask_sb = const.tile([KC, G * TQ], BF16, name="mask_sb")
        vaug = const.tile([KC, (S // KC) * GW], BF16, name="vaug")
        q_sb = [const.tile([2 * DH, S], BF16, name=f"q_sb{p}") for p in range(2)]
        k_sb = [const.tile([2 * DH, S], BF16, name=f"k_sb{p}") for p in range(2)]
        z_sb = [const.tile([2 * DH, S], BF16, name=f"z_sb{p}") for p in range(2)]

        # ---- DRAM staging for the z AllGather ----
        # bands 0..2: full-band gathers; band 3: two q-halves to cut the tail
        zb = [dram.tile([EG, TQ], BF16, name=f"zb{t}") for t in range(3)]
        zg = [dram.tile([G * EG, TQ], BF16, name=f"zg{t}") for t in range(3)]
        zbh = [dram.tile([EG, TQ // 2], BF16, name=f"zbh{i}") for i in range(2)]
        zgh = [
            dram.tile([G * EG, TQ // 2], BF16, name=f"zgh{i}") for i in range(2)
        ]
        dgi = dram.tile([1, 64], BF16, name="dgi")
        dgo = dram.tile([1, 256], BF16, name="dgo")

        # ---- prologue ----
        # dummy AllGather first: absorbs collective-stream init + rendezvous
        # cost so the first real gather is cheap
        dgz = const.tile([1, 64], BF16, name="dgz")
        nc.vector.memset(dgz[:], 0.0)
        ones_f32 = const.tile([1, DH], F32, name="ones_f32")
        nc.vector.memset(ones_f32[:], 1.0)
        nc.gpsimd.dma_start(dgi[:, :], dgz[:])
        nc.gpsimd.collective_compute(
            "AllGather",
            mybir.AluOpType.bypass,
            replica_groups=GROUPS,
            ins=[dgi.opt()],
            outs=[dgo.opt()],
        )
        # ones base for v_aug (value slots get overwritten by the v copies)
        nc.vector.memset(vaug[:], 1.0)

        x_t = {}

        def load_x_band(t):
            for nm, src in (("q", xq), ("k", xk), ("v", xv)):
                xt = xpool.tile([DC, NDC * TQ], BF16, name=f"x{nm}", tag=f"x{nm}")
                src_v = src[:, :].rearrange("(c p) s -> p c s", p=DC)[
                    :, :, t * TQ : (t + 1) * TQ
                ]
                nc.sync.dma_start(
                    xt[:].rearrange("p (c s) -> p c s", c=NDC), src_v
                )
                x_t[(nm, t)] = xt

        # band-0 activations first (they gate the first matmuls), then weights
        load_x_band(0)
        for wsrc, wdst in ((wq, wq_sb), (wk, wk_sb), (wv, wv_sb), (wo, wo_sb)):
            nc.sync.dma_start(
                wdst[:].rearrange("p (c e) -> p c e", c=NDC),
                wsrc[:, :, :].rearrange("c p e -> p c e"),
            )
        nc.sync.dma_start(mask_sb[:], mask[:, :])

        # ---- QKV projection of one band (generator: yields between quanta) ----
        def proj_band_gen(t):
            if t > 0:
                load_x_band(t)
                yield
            xq_t, xk_t, xv_t = x_t[("q", t)], x_t[("k", t)], x_t[("v", t)]
            for xt, wsb, dst in ((xq_t, wq_sb, q_sb), (xk_t, wk_sb, k_sb)):
                for p in range(2):
                    pq = prpool.tile([DC, TQ], F32, tag="pr", name="pq")
                    for c in range(NDC):
                        nc.tensor.matmul(
                            pq[:],
                            wsb[:, c * EG + p * DC : c * EG + (p + 1) * DC],
                            xt[:, c * TQ : (c + 1) * TQ],
                            start=(c == 0),
                            stop=(c == NDC - 1),
                        )
                        if c % 4 == 3:
                            yield
                    nc.vector.tensor_copy(
                        dst[p][:, t * TQ : (t + 1) * TQ], pq[:]
                    )
            for sub in range(4):
                pv = prpool.tile([DC, TQ], F32, tag="pr", name="pv")
                for c in range(NDC):
                    nc.tensor.matmul(
                        pv[:, 0:EG],
                        xv_t[:, c * TQ + sub * KC : c * TQ + (sub + 1) * KC],
                        wv_sb[:, c * EG : (c + 1) * EG],
                        start=(c == 0),
                        stop=(c == NDC - 1),
                    )
                    if c % 4 == 3:
                        yield
                kci = t * 4 + sub
                base = kci * GW
                nc.vector.tensor_copy(
                    vaug[:, base : base + GW].rearrange(
                        "p (h w) -> p h w", h=G
                    )[:, :, 0:DH],
                    pv[:, 0:EG].rearrange("p (h e) -> p h e", h=G),
                )
            yield

        # ---- output projection of one band (generator) ----
        def oproj_band_gen(t, zgt, q0, nq):
            # out[t*TQ+q0 : +nq, :] = z_all^T @ W_O[:, cols of this core]
            nqs = nq // KC
            zg_sb = zgpool.tile([DC, NDC * nq], BF16, name="zg_sb", tag="zg")
            nc.gpsimd.dma_start(
                zg_sb[:].rearrange("p (c s) -> p c s", c=NDC),
                zgt[:, :].rearrange("(c p) s -> p c s", p=DC),
            )
            yield
            o_sb = opool.tile([DC, nqs * DS], F32, name="o_sb", tag="o")
            for half in range(nqs // 2):
                po = prpool.tile([DC, 2 * DS], F32, tag="pr", name="po")
                for sub in range(2):
                    qs = half * 2 + sub
                    for c in range(NDC):
                        nc.tensor.matmul(
                            po[:, sub * DS : (sub + 1) * DS],
                            zg_sb[:, c * nq + qs * KC : c * nq + (qs + 1) * KC],
                            wo_sb[:, c * DS : (c + 1) * DS],
                            start=(c == 0),
                            stop=(c == NDC - 1),
                        )
                        if c % 4 == 3:
                            yield
                nc.vector.tensor_copy(
                    o_sb[:, half * 2 * DS : (half + 1) * 2 * DS], po[:]
                )
            nc.gpsimd.dma_start(
                out_ext[:, :].rearrange("(b p) d -> p b d", p=KC)[
                    :, 4 * t + q0 // KC : 4 * t + q0 // KC + nqs, :
                ],
                o_sb[:].rearrange("p (b d) -> p b d", b=nqs),
            )
            yield

        def pull(it, n):
            for _ in range(n):
                try:
                    next(it)
                except StopIteration:
                    return

        def drain(it):
            for _ in it:
                pass

        # ---- normalize + stage + gather a q-range of band t ----
        def make_recip_bcast(pz):
            den_r = rpool.tile([1, G * TQ], F32, tag="denr", name="den_r")
            bc = rpool.tile([DH, G * TQ], F32, tag="bc", name="bc")
            # 1/den = exp(-ln(den)) on the ACT engine: both functions live in
            # the natural_log_exp_and_others table set (no set switching), and
            # the [1, 2048] single-lane op costs ~2us/pass vs 13us for the
            # single-lane DVE reciprocal.
            ld = rpool.tile([1, G * TQ], F32, tag="ld", name="ld")
            nc.scalar.activation(ld[:], pz[DH : DH + 1, :], LN)
            nc.scalar.activation(den_r[:], ld[:], EXP, scale=-1.0)
            if USE_GPSIMD_BCAST:
                nc.gpsimd.partition_broadcast(bc[:], den_r[:], channels=DH)
            else:
                # rank-1 broadcast of 1/den on the PE
                for h in range(G):
                    pb = prpool.tile([DC, TQ], F32, tag="pr", name="pb")
                    nc.tensor.matmul(
                        pb[0:DH, :],
                        ones_f32[:],
                        den_r[:, h * TQ : (h + 1) * TQ],
                        start=True,
                        stop=True,
                    )
                    nc.vector.tensor_copy(bc[:, h * TQ : (h + 1) * TQ], pb[0:DH, :])
            return bc

        def norm_stage_gather(t, pz, bc, q0, nq, zbt, zgt):
            for h in range(G):
                p_i, off = h // 2, (h % 2) * DH
                nc.vector.tensor_mul(
                    z_sb[p_i][off : off + DH, t * TQ + q0 : t * TQ + q0 + nq],
                    pz[0:DH, h * TQ + q0 : h * TQ + q0 + nq],
                    bc[:, h * TQ + q0 : h * TQ + q0 + nq],
                )
            for p in range(2):
                nc.gpsimd.dma_start(
                    zbt[p * KC : (p + 1) * KC, :],
                    z_sb[p][:, t * TQ + q0 : t * TQ + q0 + nq],
                )
            nc.gpsimd.collective_compute(
                "AllGather",
                mybir.AluOpType.bypass,
                replica_groups=GROUPS,
                ins=[zbt.opt()],
                outs=[zgt.opt()],
            )

        # ---- attention band with interleaved background work ----
        def attention_band(t, work):
            nkc = 4 * (t + 1)
            pz = pzpool.tile([VW, G * TQ], F32, tag="pz", name="pz")
            for kci in range(nkc):
                dc = kci - 4 * t
                qv = KC * dc if dc >= 0 else 0
                nq = TQ - qv
                es = []
                for pair in range(2):
                    sc = scpool.tile([KC, 2 * TQ], F32, tag="sc", name="sc")
                    for h2 in range(2):
                        nc.tensor.matmul(
                            sc[:, h2 * TQ + qv : (h2 + 1) * TQ],
                            k_sb[pair][
                                h2 * DH : (h2 + 1) * DH,
                                kci * KC : (kci + 1) * KC,
                            ],
                            q_sb[pair][
                                h2 * DH : (h2 + 1) * DH,
                                t * TQ + qv : (t + 1) * TQ,
                            ],
                            start=True,
                            stop=True,
                            tile_position=(h2 * DH, 0),
                        )
                    e = epool.tile([KC, 2 * TQ], BF16, tag=f"e{pair}", name="e")
                    sc_v = sc[:].rearrange("p (h q) -> p h q", h=2)[:, :, qv:TQ]
                    e_v = e[:].rearrange("p (h q) -> p h q", h=2)[:, :, qv:TQ]
                    nc.scalar.activation(e_v, sc_v, EXP)
                    if dc >= 0:
                        m_v = (
                            mask_sb[:, dc * TQ + qv : (dc + 1) * TQ]
                            .unsqueeze(1)
                            .broadcast_to([KC, 2, nq])
                        )
                        nc.vector.tensor_mul(e_v, e_v, m_v)
                    es.append(e)
                for h in range(G):
                    e = es[h // 2]
                    h2 = h % 2
                    nc.tensor.matmul(
                        pz[:, h * TQ + qv : (h + 1) * TQ],
                        vaug[:, kci * GW + h * VW : kci * GW + (h + 1) * VW],
                        e[:, h2 * TQ + qv : (h2 + 1) * TQ],
                        start=(kci == 0),
                        stop=(kci == nkc - 1),
                    )
                pull(work, 2)
            return pz

        # ---- the pipeline ----
        drain(proj_band_gen(0))
        ogens = {}
        for t in range(NQT):
            gens = []
            if t + 1 < NQT:
                gens.append(proj_band_gen(t + 1))
            if t - 1 >= 0:
                gens.append(ogens[t - 1])
            work = _chain(gens)
            pz = attention_band(t, work)
            drain(work)
            bc = make_recip_bcast(pz)
            if t < 3:
                norm_stage_gather(t, pz, bc, 0, TQ, zb[t], zg[t])
                ogens[t] = oproj_band_gen(t, zg[t], 0, TQ)
            else:
                for i in range(2):
                    norm_stage_gather(
                        t, pz, bc, i * (TQ // 2), TQ // 2, zbh[i], zgh[i]
                    )
                for i in range(2):
                    drain(oproj_band_gen(t, zgh[i], i * (TQ // 2), TQ // 2))

    nc.compile()
    return nc


def _chain(gens):
    for g in gens:
        yield from g


def _get_graph() -> bass.Bass:
    if "nc" not in _CACHE:
        _CACHE["nc"] = _build()
    return _CACHE["nc"]


def _make_mask() -> np.ndarray:
    import ml_dtypes

    m = np.empty((KC, G * TQ), np.float32)
    x = np.arange(KC)[:, None]
    y = np.arange(TQ)[None, :]
    for dc in range(G):
        m[:, dc * TQ : (dc + 1) * TQ] = (dc * KC + x <= y).astype(np.float32)
    return m.astype(ml_dtypes.bfloat16)


def _make_in_maps(inputs: dict) -> list[dict]:
    import ml_dtypes

    bf16 = ml_dtypes.bfloat16
    qx = np.asarray(inputs["query_input"], np.float32).astype(bf16)
    kx = np.asarray(inputs["key_input"], np.float32).astype(bf16)
    vx = np.asarray(inputs["value_input"], np.float32).astype(bf16)
    # fold the attention scale into W_Q on the host
    WQ = (np.asarray(inputs["W_Q"], np.float32) / SCALE).astype(bf16)
    WK = np.asarray(inputs["W_K"], np.float32).astype(bf16)
    WV = np.asarray(inputs["W_V"], np.float32).astype(bf16)
    WO = np.asarray(inputs["W_O"], np.float32).astype(bf16)

    mask = _make_mask()
    xT = {
        (nm, b): np.ascontiguousarray(arr[b].T)
        for nm, arr in (("xq", qx), ("xk", kx), ("xv", vx))
        for b in range(B)
    }
    WO_flat = WO.reshape(H * DH, D)  # e' = h*64 + e, h-major (AllGather order)
    wmaps = []
    for g in range(G):
        hs = slice(g * G, (g + 1) * G)

        def prep(w):
            return np.ascontiguousarray(
                w[hs].transpose(1, 0, 2).reshape(D, EG).reshape(NDC, DC, EG)
            )

        wmaps.append(
            {
                "wq": prep(WQ),
                "wk": prep(WK),
                "wv": prep(WV),
                "wo": np.ascontiguousarray(
                    WO_flat[:, g * DS : (g + 1) * DS].reshape(NDC, DC, DS)
                ),
            }
        )

    in_maps = []
    for core in range(NCORES):
        b, g = core // G, core % G
        m = {
            "xq": xT[("xq", b)],
            "xk": xT[("xk", b)],
            "xv": xT[("xv", b)],
            "mask": mask,
        }
        m.update(wmaps[g])
        in_maps.append(m)
    return in_maps


def _assemble(results: list[dict]) -> np.ndarray:
    out = np.empty((B, S, D), np.float32)
    for core in range(NCORES):
        b, g = core // G, core % G
        out[b, :, g * DS : (g + 1) * DS] = results[core]["out"]
    return out


def run(inputs: dict, trace: bool = False):
    """Run on hardware; returns (output, BassKernelResults)."""
    nc = _get_graph()
    res = run_bass_kernel_spmd(
        nc, _make_in_maps(inputs), core_ids=list(range(NCORES)), trace=trace
    )
    return _assemble(res.results), res


def kernel(**inputs) -> np.ndarray:
    out, _ = run(inputs)
    return out
